# revision 48
# baseline (speedup 1.0000x reference)
"""Trainium2 Bass kernel for CausalWanSelfAttention (block-causal window attention).

Geometry: B=1, S=6240, DIM=1536, H=12 heads x D=128, frames of L=1560 tokens,
window = current + previous frame.

Sharding over 8 NeuronCores (sequence-parallel with KV AllGather):
  - core c owns tokens [780c, 780c+780): computes fused QKV for them
    (weights replicated), full-dim RMSNorm + RoPE locally,
  - AllGathers normed/roped K (feature-major [1536,780]) and V
    (token-major [780,1536]) across cores in bf16 (fp8 was tried and
    rejected: attention output is an incoherent weighted sum, so input
    quantization shows up ~1:1 in the final output),
  - attends its 780 queries to its 2-frame KV window (3120 tokens) read from
    the gathered buffers at per-core dynamic offsets. Frame-0 cores use a
    duplicated-frame window (softmax over a duplicated key set equals softmax
    over the single set exactly), so no masking is needed anywhere,
  - local output projection (all heads of a token live on one core).

Schedule: a dummy warmup collective fires first (absorbs the ~40us cold-start
of the collective firmware), then the V path (no norm -> its gathers fire
earliest), then K, then Q; all gather triggers fire in one batch after the
k rope (mid-loop triggers block the gpsimd queue's later rope swaps), and
ncfw executes them in data-ready order while attention heads unlock
progressively.

Layouts: q,k are feature-major bf16 [d, token]; v is token-major bf16 so it
is the stationary operand of the PV matmul directly. Head-dim order of q,k
is de-interleaved on the host (even rotary lanes first) so RoPE works on
contiguous partition halves.

Precision: matmul operands bf16 (fp32 PSUM accumulation); RMSNorm statistics
and softmax normalization in fp32; RoPE tables and products bf16 (2x DVE
rate). The norm gain g is applied via the ACT-engine per-partition scale
during PSUM evacuation; the per-token inverse-RMS is folded into the RoPE
cos/sin tables. Softmax denominator: exp chunks are pair-added on the Vector
engine (level 1 rides the QK stream) and reduced over partitions with
accumulating matmuls.

Attention is emitted as interleaved chunk streams: head h's QK+exp chunks
interleave with head h-1's PV chunks (PV leads by 4), so the PE stays busy
through the exp-paced QK stretch and the ACT engine never idles during PV.
Each head's softmax tail (denominator reciprocal -> PE broadcast -> final
normalize) is deferred past the next head's stream. Engine-queue placement
is deliberate: scalar = pure ACT compute; gpsimd = rope swaps, collectives,
v-window loads; sync = DRAM stores + k-window loads.
"""

import ml_dtypes
import numpy as np

import concourse.bass as bass
import concourse.bacc as bacc
import concourse.mybir as mybir
import concourse.tile as tile
from concourse import bass_utils

F32 = mybir.dt.float32
BF16 = mybir.dt.bfloat16
FP8 = mybir.dt.float8e4
U32 = mybir.dt.uint32
AF = mybir.ActivationFunctionType
ALU = mybir.AluOpType
NP_BF16 = ml_dtypes.bfloat16

# Geometry (hardcoded per the problem spec).
S, DIM, H, D = 6240, 1536, 12, 128
HD = H * D                      # 1536
L = 1560                        # frame length
NCORES = 8
T = S // NCORES                 # 780 tokens per core
QG = 390                        # query/token group: 2 per core, fits one PSUM bank
EPS = 1e-6
KQ = DIM // 128                 # 12 contraction chunks for the QKV matmuls
# token sub-tiles within a 780-token rank block: 6x128 + 1x12
TOK_SPLITS = [(i * 128, min(128, T - i * 128)) for i in range((T + 127) // 128)]
N_KC = 25                       # key chunks in the 3120-token window (24x128+48)


def _build_nc():
    nc = bacc.Bacc("TRN2", target_bir_lowering=False, debug=False,
                   enable_asserts=True, num_devices=NCORES)

    # ---- per-core inputs ----
    hidT = nc.dram_tensor("hidT", [DIM + 1, T], BF16, kind="ExternalInput").ap()
    csd = nc.dram_tensor("csd", [128, 2 * T], BF16, kind="ExternalInput").ap()
    wink = nc.dram_tensor("wink", [1, 4], U32, kind="ExternalInput").ap()  # 384*w
    winv = nc.dram_tensor("winv", [1, 4], U32, kind="ExternalInput").ap()  # 780*w

    # ---- replicated inputs ----
    WqkT = nc.dram_tensor("WqkT", [DIM, 2 * HD], BF16, kind="ExternalInput").ap()
    WvTa = nc.dram_tensor("WvTa", [DIM + 1, HD], BF16, kind="ExternalInput").ap()
    bqk = nc.dram_tensor("bqk", [128, 2 * H], F32, kind="ExternalInput").ap()
    bqkg = nc.dram_tensor("bqkg", [128, 2 * H], F32, kind="ExternalInput").ap()
    gcol = nc.dram_tensor("gcol", [128, 2 * H], F32, kind="ExternalInput").ap()
    WoT = nc.dram_tensor("WoT", [HD, DIM], BF16, kind="ExternalInput").ap()
    bo = nc.dram_tensor("bo", [128, DIM // 128], F32, kind="ExternalInput").ap()

    # ---- output (feature-major; host transposes back) ----
    outT = nc.dram_tensor("outT", [DIM, T], F32, kind="ExternalOutput").ap()

    # ---- internal DRAM for the collectives (fp8; pipelined so attention
    # heads unlock progressively) ----
    wrm = nc.dram_tensor("wrm", [1, 64], BF16)
    wrmg = nc.dram_tensor("wrmg", [NCORES, 64], BF16, addr_space="Shared")
    # fine-grained gathers: K per head (12), V per 256-col half (6), so the
    # serial collective channel can run K pieces the moment they are ready
    # (ncfw picks the earliest-emitted READY collective; big V gathers would
    # otherwise occupy the channel exactly when k0 lands)
    kcon = [nc.dram_tensor(f"kcon{m}", [128, T], BF16) for m in range(H)]
    vcon = [[nc.dram_tensor(f"vcon{o}_{hf}", [T, 256], BF16)
             for hf in range(2)] for o in range(3)]
    gk = [nc.dram_tensor(f"gk{m}", [NCORES * 128, T], BF16,
                         addr_space="Shared") for m in range(H)]
    gv = [[nc.dram_tensor(f"gv{o}_{hf}", [NCORES * T, 256], BF16,
                          addr_space="Shared") for hf in range(2)]
          for o in range(3)]

    with tile.TileContext(nc) as tc:
        _emit(nc, tc, hidT, csd, wink, winv, WqkT, WvTa, bqk, bqkg, gcol,
              WoT, bo, outT, kcon, vcon, gk, gv, wrm, wrmg)
    nc.compile()
    return nc


def _emit(nc, tc, hidT, csd, wink, winv, WqkT, WvTa, bqk, bqkg, gcol,
          WoT, bo, outT, kcon, vcon, gk, gv, wrm, wrmg):
    # window base registers (element offsets into gk / gv axis 0)
    kregs, vregs = [], []
    for i in range(4):
        rk = nc.alloc_registers(f"wk{i}")
        nc.regs_load(rk, wink.tensor[0:1, i:i + 1])
        kregs.append(nc.snap(rk, donate=True, min_val=0,
                             max_val=(NCORES - 1) * 128))
        rv = nc.alloc_registers(f"wv{i}")
        nc.regs_load(rv, winv.tensor[0:1, i:i + 1])
        vregs.append(nc.snap(rv, donate=True, min_val=0,
                             max_val=(NCORES - 1) * T))

    GS = (slice(0, QG), slice(QG, 2 * QG))        # token groups in SBUF
    PS2 = (slice(0, QG), slice(512, 512 + QG))    # the two bank-aligned halves

    def act2(out_sb, ps2, func, bias=0.0, scale=1.0):
        """One ACT op over both 390-wide halves of a 2-bank PSUM tile."""
        nc.scalar.activation(
            out_sb.rearrange("p (a b) -> p a b", a=2),
            ps2.rearrange("p (a b) -> p a b", a=2)[:, :, 0:QG],
            func, bias=bias, scale=scale)

    with (
        tc.tile_pool(name="const", bufs=1) as const,
        tc.tile_pool(name="qsb", bufs=1) as q_pool,       # roped q (bf16)
        tc.tile_pool(name="attsb", bufs=1) as att_pool,   # attn out
        tc.tile_pool(name="kwin", bufs=2) as kv_pool,     # gathered k windows
        tc.tile_pool(name="vwin", bufs=3) as vt_pool,     # gathered v windows
    ):
        def load_kv(h):
            """Issue the gathered-KV window loads for head h (sync queue:
            it is idle in this phase and the gather-completion waits must
            not block the compute engines' queues)."""
            ksb = kv_pool.tile([128, 4 * T], BF16, tag="ksb")
            for w in range(4):
                nc.sync.dma_start(
                    ksb[:, w * T:(w + 1) * T],
                    gk[h][bass.ds(kregs[w], 128), :])
            gvh = gv[h // 4][(h % 4) // 2]
            ho = 128 * (h % 2)
            vwin = vt_pool.tile([128, 25 * 128], BF16, tag="vwin")
            for w in range(4):
                lo = 780 * w          # window-space start of this block
                s = lo
                while s < lo + 780:
                    off = s % 128
                    if off:
                        n = min(128 - off, lo + 780 - s)
                    else:
                        n = lo + 780 - s
                    blk = s // 128
                    if off == 0 and n >= 128:
                        nb = n // 128
                        nc.gpsimd.dma_start(
                            vwin[:, 128 * blk:128 * (blk + nb)].rearrange(
                                "p (c d) -> p c d", d=128),
                            gvh[bass.ds(vregs[w] + (s - lo), 128 * nb),
                                ho:ho + 128].rearrange(
                                    "(c p) d -> p c d", p=128))
                        s += 128 * nb
                    else:
                        n = min(n, 128 - off)
                        nc.gpsimd.dma_start(
                            vwin[off:off + n, 128 * blk:128 * (blk + 1)],
                            gvh[bass.ds(vregs[w] + (s - lo), n),
                                ho:ho + 128])
                        s += n
            return ksb, vwin

        kv_list = [None] * H
        # warmup collective: absorbs the cold-start latency of the
        # collective firmware so the real gathers fire immediately.
        wu = const.tile([1, 64], BF16)
        nc.vector.memset(wu, 0.0)
        nc.gpsimd.dma_start(wrm.ap(), wu)
        nc.gpsimd.collective_compute(
            "AllGather", ALU.bypass, replica_groups=[list(range(NCORES))],
            ins=[wrm.ap()], outs=[wrmg.ap()])

        ones_col = const.tile([128, 1], F32)          # fp32 ones (norm reduce)
        nc.vector.memset(ones_col, 1.0)
        ones_bf = const.tile([128, 1], BF16)          # bf16 ones (denominator)
        nc.vector.memset(ones_bf, 1.0)
        ones_row = const.tile([1, 128], F32)          # partition-broadcast lhsT
        nc.vector.memset(ones_row, 1.0)
        ones_row_bf = const.tile([1, 128], BF16)      # bf16 broadcast lhsT
        nc.vector.memset(ones_row_bf, 1.0)
        bqk_sb = const.tile([128, 2 * H], F32)
        nc.sync.dma_start(bqk_sb, bqk)
        bqkg_sb = const.tile([128, 2 * H], F32)
        nc.sync.dma_start(bqkg_sb, bqkg)
        gcol_sb = const.tile([128, 2 * H], F32)
        nc.sync.dma_start(gcol_sb, gcol)
        bo_sb = const.tile([128, DIM // 128], F32)
        nc.sync.dma_start(bo_sb, bo)
        eps_q = const.tile([1, 1], F32)
        nc.vector.memset(eps_q, D * EPS)
        eps_k = const.tile([1, 1], F32)
        nc.vector.memset(eps_k, EPS)

        # ================= phase A: QKV projections, norms, rope, gathers ====
        with (
            tc.tile_pool(name="hid", bufs=1) as hid_pool,
            tc.tile_pool(name="wls", bufs=1) as wl_pool,
            tc.tile_pool(name="vws", bufs=2) as vw_pool,
            tc.tile_pool(name="wrk", bufs=1) as wrk_pool,
            tc.tile_pool(name="kf8", bufs=2) as kf8_pool,
            tc.tile_pool(name="tmp", bufs=2) as tmp_pool,
            tc.tile_pool(name="ropet", bufs=2) as rope_pool,
            tc.tile_pool(name="small", bufs=1) as small_pool,
            tc.tile_pool(name="csp", bufs=1) as cs_pool,
            tc.tile_pool(name="qkps", bufs=3, space="PSUM") as ps_pool,
            tc.tile_pool(name="invps", bufs=1, space="PSUM") as inv_ps_pool,
            tc.tile_pool(name="redps", bufs=1, space="PSUM") as red_ps_pool,
        ):
            # DMA issue order matters: hid first (gates the first v matmul),
            # cos/sin next (small), v weights stream inside the og loop, and
            # the big q/k weight load is issued after them (needed later).
            hid_all = hid_pool.tile([128, KQ * T], BF16, tag="hid_all")
            nc.sync.dma_start(
                hid_all.rearrange("p (c t) -> p c t", c=KQ),
                hidT.tensor[0:DIM, :].rearrange("(c p) t -> p c t", p=128))
            hid = [hid_all[:, i * T:(i + 1) * T] for i in range(KQ)]
            hid_ones = hid_pool.tile([1, T], BF16, tag="hid_ones")
            nc.sync.dma_start(hid_ones, hidT.tensor[DIM:DIM + 1, :])

            # [cos;cos] in cols 0:T, [sin;-sin] in cols T:2T (bf16: rope
            # tables only scale q/k, 0.4% rounding is inside budget)
            cs_sb = cs_pool.tile([128, 2 * T], BF16)
            nc.sync.dma_start(cs_sb, csd)



            # ---- v first: token-major, contraction over dim chunks + bias row;
            # its gathers need no norm so they fire earliest. vw_pool bufs=2:
            # the next og group's weights stream in while this one computes. ----
            for og in range(HD // 512):
                vb = tmp_pool.tile([1, 512], BF16, tag="vb")
                nc.sync.dma_start(
                    vb, WvTa.tensor[DIM:DIM + 1, 512 * og:512 * (og + 1)])
                vw_all = vw_pool.tile([128, KQ * 512], BF16, tag="vw_all")
                nc.sync.dma_start(
                    vw_all.rearrange("p (c m) -> p c m", c=KQ),
                    WvTa.tensor[0:DIM, 512 * og:512 * (og + 1)].rearrange(
                        "(c p) m -> p c m", p=128))
                vw = [vw_all[:, kc * 512:(kc + 1) * 512] for kc in range(KQ)]
                for (t0, tn_) in TOK_SPLITS:
                    ps = ps_pool.tile([128, 1024], F32, tag="qkps")
                    for kc in range(KQ):
                        nc.tensor.matmul(ps[0:tn_, 0:512],
                                         hid[kc][:, t0:t0 + tn_],
                                         vw[kc], start=(kc == 0), stop=False)
                    nc.tensor.matmul(ps[0:tn_, 0:512], hid_ones[:, t0:t0 + tn_],
                                     vb, start=False, stop=True)
                    vsb = tmp_pool.tile([128, 512], BF16, tag="vsb")
                    nc.scalar.activation(vsb[0:tn_, :], ps[0:tn_, 0:512],
                                         AF.Identity)
                    for hf in range(2):
                        nc.sync.dma_start(
                            vcon[og][hf].ap()[t0:t0 + tn_, :],
                            vsb[0:tn_, 256 * hf:256 * (hf + 1)])
                if og == 0:
                    # gv0 fires now; the other v pieces are emitted inside
                    # the k-gather batch so the channel prefers ready K work
                    for hf in range(2):
                        nc.gpsimd.collective_compute(
                            "AllGather", ALU.bypass,
                            replica_groups=[list(range(NCORES))],
                            ins=[vcon[0][hf].ap()], outs=[gv[0][hf].ap()])

            def qk_path(which, dest_for, chunk_done=None):
                mlo = H if which == "k" else 0
                # q-path rope swaps issue from the scalar queue: the gpsimd
                # queue at that point holds the gather-trigger batch whose
                # input-ready waits would delay them past q0's need time.
                swap_eng = nc.gpsimd if which == "k" else nc.scalar
                # per-path weight halves (heads 0-5, 6-11): each path streams
                # its own 2.3 MB while the previous compute runs, and the two
                # tags rotate so the q path's loads overlap the k projections.
                off = HD if which == "k" else 0
                HH = HD // 2
                whalves = []
                for hf in range(2):
                    wt = wl_pool.tile([128, KQ * HH], BF16, tag=f"wq{hf}")
                    nc.sync.dma_start(
                        wt.rearrange("p (c m) -> p c m", c=KQ),
                        WqkT.tensor[:, off + hf * HH:
                                    off + (hf + 1) * HH].rearrange(
                            "(c p) m -> p c m", p=128))
                    whalves.append(wt)
                # --- projection + biased/gained evac + sum of squares ---
                ssq = small_pool.tile([128, T], F32, tag="ssq")
                works = []
                for mi in range(H):
                    m = mlo + mi
                    work = wrk_pool.tile([128, T], BF16, tag=f"work{mi}",
                                          name=f"work{mi}")
                    works.append(work)
                    tsq = tmp_pool.tile([128, T], F32, tag="tsq")
                    ps2 = ps_pool.tile([128, 1024], F32, tag="qkps")
                    for kc in range(KQ):
                        wc = whalves[mi // 6][:, kc * HH + 128 * (mi % 6):
                                              kc * HH + 128 * (mi % 6 + 1)]
                        for g in range(2):
                            nc.tensor.matmul(ps2[:, PS2[g]], wc,
                                             hid[kc][:, GS[g]],
                                             start=(kc == 0),
                                             stop=(kc == KQ - 1))
                    # work = g * (x + b): scale applies before bias, so the
                    # bias table is pre-multiplied by g on the host.
                    act2(work, ps2, AF.Identity, bias=bqkg_sb[:, m:m + 1],
                         scale=gcol_sb[:, m:m + 1])
                    act2(tsq, ps2, AF.Square, bias=bqk_sb[:, m:m + 1])
                    if mi == 0:
                        nc.vector.tensor_copy(ssq, tsq)
                    else:
                        nc.vector.tensor_tensor(ssq, ssq, tsq, ALU.add)
                # --- rms scale: s = 1/sqrt(mean+eps)  (x 1/sqrt(D) for q) ---
                sq_scale = (D / DIM) if which == "q" else (1.0 / DIM)
                sq_bias = eps_q if which == "q" else eps_k
                inv = small_pool.tile([1, T], F32, tag="inv")
                rt = small_pool.tile([1, T], F32, tag="rt")
                for g in range(2):
                    red = red_ps_pool.tile([1, QG], F32, tag="redps")
                    nc.tensor.matmul(red, ones_col, ssq[:, GS[g]], start=True,
                                     stop=True)
                    nc.scalar.activation(rt[:, GS[g]], red, AF.Sqrt,
                                         bias=sq_bias, scale=sq_scale)
                nc.vector.reciprocal_approx_fast(inv, rt)
                # --- fold inv into the rope tables: one broadcast per path ---
                csi_c = small_pool.tile([128, T], BF16, tag="csic")
                csi_s = small_pool.tile([128, T], BF16, tag="csis")
                for g in range(2):
                    ibp = inv_ps_pool.tile([128, QG], F32, tag="invbc",
                                           name=f"invbc{g}")
                    nc.tensor.matmul(ibp, ones_row, inv[:, GS[g]],
                                     start=True, stop=True)
                    nc.vector.tensor_tensor(
                        csi_c[:, GS[g]], cs_sb[:, g * QG:(g + 1) * QG],
                        ibp, ALU.mult)
                    nc.vector.tensor_tensor(
                        csi_s[:, GS[g]], cs_sb[:, T + g * QG:T + (g + 1) * QG],
                        ibp, ALU.mult)
                # --- rope -> dest, full-width bf16 (2x DVE rate), per head ---
                for mi in range(H):
                    work = works[mi]
                    dest = dest_for(mi)
                    ta = rope_pool.tile([128, T], BF16, tag="ra")
                    tb = rope_pool.tile([128, T], BF16, tag="rb")
                    sw = rope_pool.tile([128, T], BF16, tag="rsw")
                    nc.vector.tensor_tensor(ta, work, csi_c, ALU.mult)
                    nc.vector.tensor_tensor(tb, work, csi_s, ALU.mult)
                    swap_eng.dma_start(sw[0:64, :], tb[64:128, :])
                    swap_eng.dma_start(sw[64:128, :], tb[0:64, :])
                    nc.vector.tensor_tensor(dest, ta, sw, ALU.add)
                    if chunk_done is not None:
                        chunk_done(mi, dest)

            # ---- k second (feeds the remaining collectives); fp8 dest
            # tiles rotate through a 4-deep pool (k lives on in DRAM) ----
            def k_dest(mi):
                return kf8_pool.tile([128, T], BF16, tag="k8", name=f"kt{mi}")

            def k_chunk_done(mi, dest):
                nc.sync.dma_start(kcon[mi].ap(), dest)

            qk_path("k", k_dest, k_chunk_done)

            # all remaining gather triggers fire in one batch AFTER the rope
            # loop: a trigger placed mid-loop blocks the later heads' rope
            # swaps on the gpsimd queue (the input-ready wait), and ncfw runs
            # collectives by earliest-emitted-ready, so the order here is the
            # channel's priority order: early k heads first, v pieces placed
            # where attention will need them.
            def ag(tin, tout):
                nc.gpsimd.collective_compute(
                    "AllGather", ALU.bypass,
                    replica_groups=[list(range(NCORES))],
                    ins=[tin.ap()], outs=[tout.ap()])

            for m in range(4):
                ag(kcon[m], gk[m])
            ag(vcon[1][0], gv[1][0])
            ag(vcon[1][1], gv[1][1])
            for m in range(4, 8):
                ag(kcon[m], gk[m])
            ag(vcon[2][0], gv[2][0])
            ag(vcon[2][1], gv[2][1])
            for m in range(8, H):
                ag(kcon[m], gk[m])

            # preload the first attention KV window: issued behind the
            # kcon stores on the sync queue, the transfer lands as the
            # gathers complete, and attention starts the moment q0 is roped.
            kv_list[0] = load_kv(0)

            # ---- q last ----
            q_tiles = [q_pool.tile([128, T], BF16, tag=f"q{h}", name=f"qt{h}")
                       for h in range(H)]
            qk_path("q", lambda mi: q_tiles[mi])

        # ================= phase B: attention ================================
        with (
            tc.tile_pool(name="probs", bufs=28) as probs_pool,
            tc.tile_pool(name="pairs", bufs=16) as pair_pool,
            tc.tile_pool(name="attm", bufs=2) as attm_pool,
            tc.tile_pool(name="attsc", bufs=2, space="PSUM") as sc_ps,
            tc.tile_pool(name="attop", bufs=1, space="PSUM") as out_ps,
            tc.tile_pool(name="attden", bufs=1, space="PSUM") as den_ps,
        ):
            att_tiles = []

            def emit_tail(st):
                """Deferred per-head softmax tail: evac, recip, bcast, mult."""
                op2, dps, ath = st
                osb = attm_pool.tile([128, 2 * QG], F32, tag="osb")
                act2(osb, op2, AF.Identity)
                dsb = attm_pool.tile([1, 2 * QG], F32, tag="dsb")
                for g in range(2):
                    nc.vector.reciprocal_approx_fast(dsb[:, GS[g]], dps[g])
                dsb_bf = attm_pool.tile([1, 2 * QG], BF16, tag="dsbb")
                nc.vector.tensor_copy(dsb_bf, dsb)
                bc2 = out_ps.tile([128, 1024], F32, tag="op")
                for g in range(2):
                    nc.tensor.matmul(bc2[:, PS2[g]], ones_row_bf,
                                     dsb_bf[:, GS[g]], start=True, stop=True)
                nc.vector.tensor_tensor(
                    ath.rearrange("p (a b) -> p a b", a=2),
                    osb.rearrange("p (a b) -> p a b", a=2),
                    bc2.rearrange("p (a b) -> p a b", a=2)[:, :, 0:QG],
                    ALU.mult)

            def pv_chunk(op2t, vwin_, prs_, ci):
                cn, pr = prs_[ci]
                vt = vwin_[:, 128 * ci:128 * (ci + 1)]
                for g in range(2):
                    nc.tensor.matmul(op2t[:, PS2[g]], vt[0:cn, :],
                                     pr[0:cn, GS[g]],
                                     start=(ci == 0), stop=(ci == N_KC - 1))

            def dps_reduce(dpst, ppart):
                for g in range(2):
                    for j, (pn, pt) in enumerate(ppart):
                        nc.tensor.matmul(dpst[g], ones_bf[0:pn, :],
                                         pt[0:pn, GS[g]],
                                         start=(j == 0), stop=(j == 1))

            def start_pv(prev_):
                """Allocate the PV accumulator + denominator for head h-1 and
                emit its first 4 PV chunks (PV leads QK by 4 in the interleave
                so the probs pool rotation never waits on a future reader)."""
                pprs, ppart, pvwin, path_ = prev_
                pop2 = out_ps.tile([128, 1024], F32, tag="op")
                pdps = [den_ps.tile([1, QG], F32, tag="dp0", name="dp0"),
                        den_ps.tile([1, QG], F32, tag="dp1", name="dp1")]
                for ci in range(4):
                    pv_chunk(pop2, pvwin, pprs, ci)
                return pop2, pdps

            prev = None          # (prs, partials, vwin, ath) of head h-1
            for h in range(H):
                ksb, vwin = kv_list[h]
                if h + 1 < H:
                    kv_list[h + 1] = load_kv(h + 1)
                ath = att_pool.tile([128, T], BF16, tag=f"att{h}")
                att_tiles.append(ath)
                if prev is not None:
                    pop2, pdps = start_pv(prev)
                # interleaved stream: this head's QK+exp chunks with the
                # previous head's PV chunks, so the PE stays busy during the
                # exp-paced QK stretch and the ACT engine never waits on PV.
                prs = []
                lvl = []
                for ci in range(N_KC):
                    c0 = 128 * ci
                    cn = min(128, 4 * T - c0)          # window is 3120 tokens
                    sp2 = sc_ps.tile([128, 1024], F32, tag="sp")
                    for g in range(2):
                        nc.tensor.matmul(
                            sp2[0:cn, PS2[g]], ksb[:, c0:c0 + cn],
                            q_tiles[h][:, GS[g]], start=True, stop=True)
                    pr = probs_pool.tile([128, 2 * QG], BF16, tag="pr")
                    act2(pr[0:cn, :], sp2[0:cn, :], AF.Exp)
                    prs.append((cn, pr))
                    # level-1 of the denominator pair tree rides the stream so
                    # the tree root is ready long before the next head's dps
                    if ci % 2 == 1 and ci < N_KC - 1:
                        pp = pair_pool.tile([128, 2 * QG], BF16, tag="pp")
                        nc.vector.tensor_tensor(pp, prs[ci - 1][1], pr,
                                                ALU.add)
                        lvl.append(pp)
                    if prev is not None:
                        if ci + 4 < N_KC:
                            pv_chunk(pop2, prev[2], prev[0], ci + 4)
                        if ci == 20:
                            dps_reduce(pdps, prev[1])
                # remaining tree levels; the 48-row tail chunk joins at the
                # matmul reduce
                while len(lvl) > 1:
                    nxt = []
                    for i in range(0, len(lvl) - 1, 2):
                        pp = pair_pool.tile([128, 2 * QG], BF16, tag="pp")
                        nc.vector.tensor_tensor(pp, lvl[i], lvl[i + 1],
                                                ALU.add)
                        nxt.append(pp)
                    if len(lvl) % 2:
                        nxt.append(lvl[-1])
                    lvl = nxt
                partials = [(128, lvl[0]), prs[N_KC - 1]]
                if prev is not None:
                    emit_tail((pop2, pdps, prev[3]))
                prev = (prs, partials, vwin, ath)
            # final head's PV has no next head to hide under; run it densely
            pop2, pdps = start_pv(prev)
            for ci in range(4, N_KC):
                pv_chunk(pop2, prev[2], prev[0], ci)
            dps_reduce(pdps, prev[1])
            emit_tail((pop2, pdps, prev[3]))

        # ================= phase C: output projection ========================
        with (
            tc.tile_pool(name="wos", bufs=3) as wo_pool,
            tc.tile_pool(name="osbp", bufs=2) as o_pool,
            tc.tile_pool(name="opps", bufs=2, space="PSUM") as op_ps,
        ):
            for od in range(DIM // 128):
                wo = wo_pool.tile([128, HD], BF16, tag="wo")
                nc.sync.dma_start(
                    wo.rearrange("p (c m) -> p c m", c=H),
                    WoT.tensor[:, 128 * od:128 * (od + 1)].rearrange(
                        "(c p) m -> p c m", p=128))
                ot = o_pool.tile([128, T], F32, tag="ot")
                ps2 = op_ps.tile([128, 1024], F32, tag="opps")
                for hc in range(H):
                    for g in range(2):
                        nc.tensor.matmul(ps2[:, PS2[g]],
                                         wo[:, 128 * hc:128 * (hc + 1)],
                                         att_tiles[hc][:, GS[g]],
                                         start=(hc == 0), stop=(hc == H - 1))
                act2(ot, ps2, AF.Identity, bias=bo_sb[:, od:od + 1])
                nc.sync.dma_start(outT.tensor[128 * od:128 * (od + 1), :], ot)


_CACHED_NC = None
_LAST_IN_MAPS = None


def _get_nc():
    global _CACHED_NC
    if _CACHED_NC is None:
        _CACHED_NC = _build_nc()
    return _CACHED_NC


def _deinterleave(n):
    """Permutation putting even rotary lanes first within each 128-dim head."""
    idx = np.arange(n).reshape(-1, D)
    return np.concatenate([idx[:, 0::2], idx[:, 1::2]], axis=1).reshape(-1)


def kernel(hidden_states, freqs_cos, freqs_sin, W_qkv, b_qkv, gq, gk, W_out,
           b_out):
    hidden_states = np.asarray(hidden_states, dtype=np.float32)
    freqs_cos = np.asarray(freqs_cos, dtype=np.float32)
    freqs_sin = np.asarray(freqs_sin, dtype=np.float32)
    W_qkv = np.asarray(W_qkv, dtype=np.float32)
    b_qkv = np.asarray(b_qkv, dtype=np.float32)
    gq = np.asarray(gq, dtype=np.float32)
    gk = np.asarray(gk, dtype=np.float32)
    W_out = np.asarray(W_out, dtype=np.float32)
    b_out = np.asarray(b_out, dtype=np.float32)

    nc = _get_nc()

    perm = _deinterleave(HD)
    Wq, Wk, Wv = W_qkv[:HD][perm], W_qkv[HD:2 * HD][perm], W_qkv[2 * HD:]
    bq, bk, bv = b_qkv[:HD][perm], b_qkv[HD:2 * HD][perm], b_qkv[2 * HD:]
    gqp, gkp = gq[perm], gk[perm]

    WqkT = np.ascontiguousarray(
        np.concatenate([Wq, Wk], axis=0).T).astype(NP_BF16)   # [1536, 3072]
    WvTa = np.concatenate([Wv.T, bv[None, :]],
                          axis=0).astype(NP_BF16)             # [1537, 1536]
    bqk_h = np.concatenate([bq, bk])
    g_h = np.concatenate([gqp, gkp])
    bqk_t = np.ascontiguousarray(bqk_h.reshape(2 * H, 128).T)       # [128, 24]
    bqkg_t = np.ascontiguousarray((bqk_h * g_h).reshape(2 * H, 128).T)
    gcol_t = np.ascontiguousarray(g_h.reshape(2 * H, 128).T)
    WoT = np.ascontiguousarray(W_out.T).astype(NP_BF16)       # [1536, 1536]
    bo = np.ascontiguousarray(b_out.reshape(DIM // 128, 128).T)  # [128, 12]

    in_maps = []
    for c in range(NCORES):
        sl = slice(c * T, (c + 1) * T)
        hidT = np.concatenate([
            np.ascontiguousarray(hidden_states[0, sl, :].T),
            np.ones((1, T), np.float32)], axis=0).astype(NP_BF16)  # [1537, 780]
        f = (c * T) // L
        if f == 0:
            win = [0, 1, 0, 1]
        else:
            base = 2 * (f - 1)
            win = [base, base + 1, base + 2, base + 3]
        cc = np.ascontiguousarray(freqs_cos[sl].T)            # [64, 780]
        ss = np.ascontiguousarray(freqs_sin[sl].T)
        csd = np.concatenate([
            np.concatenate([cc, cc], axis=0),
            np.concatenate([ss, -ss], axis=0)], axis=1).astype(NP_BF16)
        in_maps.append({
            "hidT": hidT,
            "csd": csd,
            "wink": np.asarray([[w * 128 for w in win]], np.uint32),
            "winv": np.asarray([[w * T for w in win]], np.uint32),
            "WqkT": WqkT, "WvTa": WvTa, "bqk": bqk_t, "bqkg": bqkg_t,
            "gcol": gcol_t, "WoT": WoT, "bo": bo,
        })

    global _LAST_IN_MAPS
    _LAST_IN_MAPS = in_maps
    res = bass_utils.run_bass_kernel_spmd(nc, in_maps,
                                          core_ids=list(range(NCORES)))
    out = np.empty((1, S, DIM), np.float32)
    for c in range(NCORES):
        out[0, c * T:(c + 1) * T, :] = res.results[c]["outT"].T
    return out


# revision 55
# speedup vs baseline: 1.0123x; 1.0123x over previous
"""Trainium2 Bass kernel for CausalWanSelfAttention (block-causal window attention).

Geometry: B=1, S=6240, DIM=1536, H=12 heads x D=128, frames of L=1560 tokens,
window = current + previous frame.

Sharding over 8 NeuronCores (sequence-parallel with KV AllGather):
  - core c owns tokens [780c, 780c+780): computes fused QKV for them
    (weights replicated), full-dim RMSNorm + RoPE locally,
  - AllGathers normed/roped K (feature-major [1536,780]) and V
    (token-major [780,1536]) across cores in bf16 (fp8 was tried and
    rejected: attention output is an incoherent weighted sum, so input
    quantization shows up ~1:1 in the final output),
  - attends its 780 queries to its 2-frame KV window (3120 tokens) read from
    the gathered buffers at per-core dynamic offsets. Frame-0 cores use a
    duplicated-frame window (softmax over a duplicated key set equals softmax
    over the single set exactly), so no masking is needed anywhere,
  - local output projection (all heads of a token live on one core).

Schedule: a dummy warmup collective fires first (absorbs the ~40us cold-start
of the collective firmware), then the V path (no norm -> its gathers fire
earliest), then K, then Q; all gather triggers fire in one batch after the
k rope (mid-loop triggers block the gpsimd queue's later rope swaps), and
ncfw executes them in data-ready order while attention heads unlock
progressively.

Layouts: q,k are feature-major bf16 [d, token]; v is token-major bf16 so it
is the stationary operand of the PV matmul directly. Head-dim order of q,k
is de-interleaved on the host (even rotary lanes first) so RoPE works on
contiguous partition halves.

Precision: matmul operands bf16 (fp32 PSUM accumulation); RMSNorm statistics
and softmax normalization in fp32; RoPE tables and products bf16 (2x DVE
rate). The norm gain g is applied via the ACT-engine per-partition scale
during PSUM evacuation; the per-token inverse-RMS is folded into the RoPE
cos/sin tables. Softmax denominator: exp chunks are pair-added on the Vector
engine (level 1 rides the QK stream) and reduced over partitions with
accumulating matmuls.

Attention is emitted as interleaved chunk streams: head h's QK+exp chunks
interleave with head h-1's PV chunks (PV leads by 4), so the PE stays busy
through the exp-paced QK stretch and the ACT engine never idles during PV.
Each head's softmax tail (denominator reciprocal -> PE broadcast -> final
normalize) is deferred past the next head's stream. Engine-queue placement
is deliberate: scalar = pure ACT compute; gpsimd = rope swaps, collectives,
v-window loads; sync = DRAM stores + k-window loads.
"""

import ml_dtypes
import numpy as np

import concourse.bass as bass
import concourse.bacc as bacc
import concourse.mybir as mybir
import concourse.tile as tile
from concourse import bass_utils

F32 = mybir.dt.float32
BF16 = mybir.dt.bfloat16
FP8 = mybir.dt.float8e4
U32 = mybir.dt.uint32
AF = mybir.ActivationFunctionType
ALU = mybir.AluOpType
NP_BF16 = ml_dtypes.bfloat16

# Geometry (hardcoded per the problem spec).
S, DIM, H, D = 6240, 1536, 12, 128
HD = H * D                      # 1536
L = 1560                        # frame length
NCORES = 8
T = S // NCORES                 # 780 tokens per core
QG = 390                        # query/token group: 2 per core, fits one PSUM bank
EPS = 1e-6
KQ = DIM // 128                 # 12 contraction chunks for the QKV matmuls
# token sub-tiles within a 780-token rank block: 6x128 + 1x12
TOK_SPLITS = [(i * 128, min(128, T - i * 128)) for i in range((T + 127) // 128)]
N_KC = 25                       # key chunks in the 3120-token window (24x128+48)


def _build_nc():
    nc = bacc.Bacc("TRN2", target_bir_lowering=False, debug=False,
                   enable_asserts=True, num_devices=NCORES)

    # ---- per-core inputs ----
    hidT = nc.dram_tensor("hidT", [DIM + 1, T], BF16, kind="ExternalInput").ap()
    csd = nc.dram_tensor("csd", [128, 2 * T], BF16, kind="ExternalInput").ap()
    wink = nc.dram_tensor("wink", [1, 4], U32, kind="ExternalInput").ap()  # 384*w
    winv = nc.dram_tensor("winv", [1, 4], U32, kind="ExternalInput").ap()  # 780*w

    # ---- replicated inputs ----
    WqkT = nc.dram_tensor("WqkT", [DIM, 2 * HD], BF16, kind="ExternalInput").ap()
    WvTa = nc.dram_tensor("WvTa", [DIM + 1, HD], BF16, kind="ExternalInput").ap()
    bqk = nc.dram_tensor("bqk", [128, 2 * H], F32, kind="ExternalInput").ap()
    bqkg = nc.dram_tensor("bqkg", [128, 2 * H], F32, kind="ExternalInput").ap()
    gcol = nc.dram_tensor("gcol", [128, 2 * H], F32, kind="ExternalInput").ap()
    WoT = nc.dram_tensor("WoT", [HD, DIM], BF16, kind="ExternalInput").ap()
    bo = nc.dram_tensor("bo", [128, DIM // 128], F32, kind="ExternalInput").ap()

    # ---- output (feature-major; host transposes back) ----
    outT = nc.dram_tensor("outT", [DIM, T], F32, kind="ExternalOutput").ap()

    # ---- internal DRAM for the collectives (fp8; pipelined so attention
    # heads unlock progressively) ----
    wrm = nc.dram_tensor("wrm", [1, 64], BF16)
    wrmg = nc.dram_tensor("wrmg", [NCORES, 64], BF16, addr_space="Shared")
    kcon = [nc.dram_tensor(f"kcon{g}", [3 * 128, T], BF16) for g in range(4)]
    vcon = [nc.dram_tensor(f"vcon{o}", [T, 512], BF16) for o in range(3)]
    gk = [nc.dram_tensor(f"gk{g}", [NCORES * 3 * 128, T], BF16,
                         addr_space="Shared") for g in range(4)]
    gv = [nc.dram_tensor(f"gv{o}", [NCORES * T, 512], BF16,
                         addr_space="Shared") for o in range(3)]

    with tile.TileContext(nc) as tc:
        _emit(nc, tc, hidT, csd, wink, winv, WqkT, WvTa, bqk, bqkg, gcol,
              WoT, bo, outT, kcon, vcon, gk, gv, wrm, wrmg)
    nc.compile()
    return nc


def _emit(nc, tc, hidT, csd, wink, winv, WqkT, WvTa, bqk, bqkg, gcol,
          WoT, bo, outT, kcon, vcon, gk, gv, wrm, wrmg):
    # window base registers (element offsets into gk / gv axis 0)
    kregs, vregs = [], []
    for i in range(4):
        rk = nc.alloc_registers(f"wk{i}")
        nc.regs_load(rk, wink.tensor[0:1, i:i + 1])
        kregs.append(nc.snap(rk, donate=True, min_val=0,
                             max_val=(NCORES - 1) * 3 * 128))
        rv = nc.alloc_registers(f"wv{i}")
        nc.regs_load(rv, winv.tensor[0:1, i:i + 1])
        vregs.append(nc.snap(rv, donate=True, min_val=0,
                             max_val=(NCORES - 1) * T))

    GS = (slice(0, QG), slice(QG, 2 * QG))        # token groups in SBUF
    PS2 = (slice(0, QG), slice(512, 512 + QG))    # the two bank-aligned halves

    def act2(out_sb, ps2, func, bias=0.0, scale=1.0):
        """One ACT op over both 390-wide halves of a 2-bank PSUM tile."""
        nc.scalar.activation(
            out_sb.rearrange("p (a b) -> p a b", a=2),
            ps2.rearrange("p (a b) -> p a b", a=2)[:, :, 0:QG],
            func, bias=bias, scale=scale)

    with (
        tc.tile_pool(name="const", bufs=1) as const,
        tc.tile_pool(name="qsb", bufs=1) as q_pool,       # roped q (bf16)
        tc.tile_pool(name="attsb", bufs=1) as att_pool,   # attn out
        tc.tile_pool(name="kwin", bufs=2) as kv_pool,     # gathered k windows
        tc.tile_pool(name="vwin", bufs=3) as vt_pool,     # gathered v windows
    ):
        def load_kv(h):
            """Issue the gathered-KV window loads for head h (sync queue:
            it is idle in this phase and the gather-completion waits must
            not block the compute engines' queues)."""
            ksb = kv_pool.tile([128, 4 * T], BF16, tag="ksb")
            for w in range(4):
                nc.sync.dma_start(
                    ksb[:, w * T:(w + 1) * T],
                    gk[h // 3][bass.ds(kregs[w] + 128 * (h % 3), 128), :])
            gvh = gv[h // 4]
            ho = 128 * (h % 4)
            vwin = vt_pool.tile([128, 25 * 128], BF16, tag="vwin")
            for w in range(4):
                lo = 780 * w          # window-space start of this block
                s = lo
                while s < lo + 780:
                    off = s % 128
                    if off:
                        n = min(128 - off, lo + 780 - s)
                    else:
                        n = lo + 780 - s
                    blk = s // 128
                    if off == 0 and n >= 128:
                        nb = n // 128
                        nc.gpsimd.dma_start(
                            vwin[:, 128 * blk:128 * (blk + nb)].rearrange(
                                "p (c d) -> p c d", d=128),
                            gvh[bass.ds(vregs[w] + (s - lo), 128 * nb),
                                ho:ho + 128].rearrange(
                                    "(c p) d -> p c d", p=128))
                        s += 128 * nb
                    else:
                        n = min(n, 128 - off)
                        nc.gpsimd.dma_start(
                            vwin[off:off + n, 128 * blk:128 * (blk + 1)],
                            gvh[bass.ds(vregs[w] + (s - lo), n),
                                ho:ho + 128])
                        s += n
            return ksb, vwin

        kv_list = [None] * H
        # warmup collective: absorbs the cold-start latency of the
        # collective firmware so the real gathers fire immediately.
        wu = const.tile([1, 64], BF16)
        nc.vector.memset(wu, 0.0)
        nc.gpsimd.dma_start(wrm.ap(), wu)
        nc.gpsimd.collective_compute(
            "AllGather", ALU.bypass, replica_groups=[list(range(NCORES))],
            ins=[wrm.ap()], outs=[wrmg.ap()])

        ones_col = const.tile([128, 1], F32)          # fp32 ones (norm reduce)
        nc.vector.memset(ones_col, 1.0)
        ones_bf = const.tile([128, 1], BF16)          # bf16 ones (denominator)
        nc.vector.memset(ones_bf, 1.0)
        ones_row = const.tile([1, 128], F32)          # partition-broadcast lhsT
        nc.vector.memset(ones_row, 1.0)
        ones_row_bf = const.tile([1, 128], BF16)      # bf16 broadcast lhsT
        nc.vector.memset(ones_row_bf, 1.0)
        bqk_sb = const.tile([128, 2 * H], F32)
        nc.sync.dma_start(bqk_sb, bqk)
        bqkg_sb = const.tile([128, 2 * H], F32)
        nc.sync.dma_start(bqkg_sb, bqkg)
        gcol_sb = const.tile([128, 2 * H], F32)
        nc.sync.dma_start(gcol_sb, gcol)
        bo_sb = const.tile([128, DIM // 128], F32)
        nc.sync.dma_start(bo_sb, bo)
        eps_q = const.tile([1, 1], F32)
        nc.vector.memset(eps_q, D * EPS)
        eps_k = const.tile([1, 1], F32)
        nc.vector.memset(eps_k, EPS)

        # ================= phase A: QKV projections, norms, rope, gathers ====
        with (
            tc.tile_pool(name="hid", bufs=1) as hid_pool,
            tc.tile_pool(name="wls", bufs=1) as wl_pool,
            tc.tile_pool(name="vws", bufs=2) as vw_pool,
            tc.tile_pool(name="wrk", bufs=1) as wrk_pool,
            tc.tile_pool(name="kf8", bufs=2) as kf8_pool,
            tc.tile_pool(name="tmp", bufs=2) as tmp_pool,
            tc.tile_pool(name="ropet", bufs=2) as rope_pool,
            tc.tile_pool(name="small", bufs=1) as small_pool,
            tc.tile_pool(name="csp", bufs=1) as cs_pool,
            tc.tile_pool(name="qkps", bufs=3, space="PSUM") as ps_pool,
            tc.tile_pool(name="invps", bufs=1, space="PSUM") as inv_ps_pool,
            tc.tile_pool(name="redps", bufs=1, space="PSUM") as red_ps_pool,
        ):
            # DMA issue order matters: hid first (gates the first v matmul),
            # cos/sin next (small), v weights stream inside the og loop, and
            # the big q/k weight load is issued after them (needed later).
            hid_all = hid_pool.tile([128, KQ * T], BF16, tag="hid_all")
            nc.sync.dma_start(
                hid_all.rearrange("p (c t) -> p c t", c=KQ),
                hidT.tensor[0:DIM, :].rearrange("(c p) t -> p c t", p=128))
            hid = [hid_all[:, i * T:(i + 1) * T] for i in range(KQ)]
            hid_ones = hid_pool.tile([1, T], BF16, tag="hid_ones")
            nc.sync.dma_start(hid_ones, hidT.tensor[DIM:DIM + 1, :])

            # [cos;cos] in cols 0:T, [sin;-sin] in cols T:2T (bf16: rope
            # tables only scale q/k, 0.4% rounding is inside budget)
            cs_sb = cs_pool.tile([128, 2 * T], BF16)
            nc.sync.dma_start(cs_sb, csd)



            # ---- v first: token-major, contraction over dim chunks + bias row;
            # its gathers need no norm so they fire earliest. vw_pool bufs=2:
            # the next og group's weights stream in while this one computes. ----
            for og in range(HD // 512):
                vb = tmp_pool.tile([1, 512], BF16, tag="vb")
                nc.sync.dma_start(
                    vb, WvTa.tensor[DIM:DIM + 1, 512 * og:512 * (og + 1)])
                vw_all = vw_pool.tile([128, KQ * 512], BF16, tag="vw_all")
                nc.sync.dma_start(
                    vw_all.rearrange("p (c m) -> p c m", c=KQ),
                    WvTa.tensor[0:DIM, 512 * og:512 * (og + 1)].rearrange(
                        "(c p) m -> p c m", p=128))
                vw = [vw_all[:, kc * 512:(kc + 1) * 512] for kc in range(KQ)]
                for (t0, tn_) in TOK_SPLITS:
                    ps = ps_pool.tile([128, 1024], F32, tag="qkps")
                    for kc in range(KQ):
                        nc.tensor.matmul(ps[0:tn_, 0:512],
                                         hid[kc][:, t0:t0 + tn_],
                                         vw[kc], start=(kc == 0), stop=False)
                    nc.tensor.matmul(ps[0:tn_, 0:512], hid_ones[:, t0:t0 + tn_],
                                     vb, start=False, stop=True)
                    vsb = tmp_pool.tile([128, 512], BF16, tag="vsb")
                    nc.scalar.activation(vsb[0:tn_, :], ps[0:tn_, 0:512],
                                         AF.Identity)
                    nc.sync.dma_start(vcon[og].ap()[t0:t0 + tn_, :],
                                      vsb[0:tn_, :])
                if og == 0:
                    # gv1/gv2 are deferred into the k-gather batch so the
                    # serial Comms channel completes gathers in head-need
                    # order: v0, k0, k1, v1, k2, v2, k3
                    nc.gpsimd.collective_compute(
                        "AllGather", ALU.bypass,
                        replica_groups=[list(range(NCORES))],
                        ins=[vcon[og].ap()], outs=[gv[og].ap()])

            def qk_path(which, dest_for, chunk_done=None):
                mlo = H if which == "k" else 0
                swap_eng = nc.gpsimd
                # per-path weight halves (heads 0-5, 6-11): each path streams
                # its own 2.3 MB while the previous compute runs, and the two
                # tags rotate so the q path's loads overlap the k projections.
                off = HD if which == "k" else 0
                HH = HD // 2
                whalves = []
                for hf in range(2):
                    wt = wl_pool.tile([128, KQ * HH], BF16, tag=f"wq{hf}")
                    nc.sync.dma_start(
                        wt.rearrange("p (c m) -> p c m", c=KQ),
                        WqkT.tensor[:, off + hf * HH:
                                    off + (hf + 1) * HH].rearrange(
                            "(c p) m -> p c m", p=128))
                    whalves.append(wt)
                # --- projection + biased/gained evac + sum of squares ---
                ssq = small_pool.tile([128, T], F32, tag="ssq")
                works = []
                for mi in range(H):
                    m = mlo + mi
                    work = wrk_pool.tile([128, T], BF16, tag=f"work{mi}",
                                          name=f"work{mi}")
                    works.append(work)
                    tsq = tmp_pool.tile([128, T], F32, tag="tsq")
                    ps2 = ps_pool.tile([128, 1024], F32, tag="qkps")
                    for kc in range(KQ):
                        wc = whalves[mi // 6][:, kc * HH + 128 * (mi % 6):
                                              kc * HH + 128 * (mi % 6 + 1)]
                        for g in range(2):
                            nc.tensor.matmul(ps2[:, PS2[g]], wc,
                                             hid[kc][:, GS[g]],
                                             start=(kc == 0),
                                             stop=(kc == KQ - 1))
                    # work = g * (x + b): scale applies before bias, so the
                    # bias table is pre-multiplied by g on the host.
                    act2(work, ps2, AF.Identity, bias=bqkg_sb[:, m:m + 1],
                         scale=gcol_sb[:, m:m + 1])
                    act2(tsq, ps2, AF.Square, bias=bqk_sb[:, m:m + 1])
                    if mi == 0:
                        nc.vector.tensor_copy(ssq, tsq)
                    else:
                        nc.vector.tensor_tensor(ssq, ssq, tsq, ALU.add)
                # --- rms scale: s = 1/sqrt(mean+eps)  (x 1/sqrt(D) for q) ---
                sq_scale = (D / DIM) if which == "q" else (1.0 / DIM)
                sq_bias = eps_q if which == "q" else eps_k
                inv = small_pool.tile([1, T], F32, tag="inv")
                rt = small_pool.tile([1, T], F32, tag="rt")
                for g in range(2):
                    red = red_ps_pool.tile([1, QG], F32, tag="redps")
                    nc.tensor.matmul(red, ones_col, ssq[:, GS[g]], start=True,
                                     stop=True)
                    nc.scalar.activation(rt[:, GS[g]], red, AF.Sqrt,
                                         bias=sq_bias, scale=sq_scale)
                nc.vector.reciprocal_approx_fast(inv, rt)
                # --- fold inv into the rope tables: one broadcast per path ---
                csi_c = small_pool.tile([128, T], BF16, tag="csic")
                csi_s = small_pool.tile([128, T], BF16, tag="csis")
                for g in range(2):
                    ibp = inv_ps_pool.tile([128, QG], F32, tag="invbc",
                                           name=f"invbc{g}")
                    nc.tensor.matmul(ibp, ones_row, inv[:, GS[g]],
                                     start=True, stop=True)
                    nc.vector.tensor_tensor(
                        csi_c[:, GS[g]], cs_sb[:, g * QG:(g + 1) * QG],
                        ibp, ALU.mult)
                    nc.vector.tensor_tensor(
                        csi_s[:, GS[g]], cs_sb[:, T + g * QG:T + (g + 1) * QG],
                        ibp, ALU.mult)
                # --- rope -> dest, full-width bf16 (2x DVE rate), per head ---
                for mi in range(H):
                    work = works[mi]
                    dest = dest_for(mi)
                    ta = rope_pool.tile([128, T], BF16, tag="ra")
                    tb = rope_pool.tile([128, T], BF16, tag="rb")
                    sw = rope_pool.tile([128, T], BF16, tag="rsw")
                    nc.vector.tensor_tensor(ta, work, csi_c, ALU.mult)
                    nc.vector.tensor_tensor(tb, work, csi_s, ALU.mult)
                    swap_eng.dma_start(sw[0:64, :], tb[64:128, :])
                    swap_eng.dma_start(sw[64:128, :], tb[0:64, :])
                    nc.vector.tensor_tensor(dest, ta, sw, ALU.add)
                    if chunk_done is not None:
                        chunk_done(mi, dest)

            # ---- k second (feeds the remaining collectives); fp8 dest
            # tiles rotate through a 4-deep pool (k lives on in DRAM) ----
            def k_dest(mi):
                return kf8_pool.tile([128, T], BF16, tag="k8", name=f"kt{mi}")

            def k_chunk_done(mi, dest):
                g = mi // 3
                nc.sync.dma_start(
                    kcon[g].ap()[128 * (mi % 3):128 * (mi % 3 + 1), :], dest)

            qk_path("k", k_dest, k_chunk_done)

            # all remaining gather triggers fire in one batch AFTER the rope
            # loop: a trigger placed mid-loop blocks the later heads' rope
            # swaps on the gpsimd queue (the input-ready wait), and ncfw
            # serializes the collectives anyway so early triggers buy nothing.
            for g in range(4):
                nc.gpsimd.collective_compute(
                    "AllGather", ALU.bypass,
                    replica_groups=[list(range(NCORES))],
                    ins=[kcon[g].ap()], outs=[gk[g].ap()])
                if g in (1, 2):     # v1 after k1, v2 after k2
                    nc.gpsimd.collective_compute(
                        "AllGather", ALU.bypass,
                        replica_groups=[list(range(NCORES))],
                        ins=[vcon[g].ap()], outs=[gv[g].ap()])

            # preload the first attention KV window: issued behind the
            # kcon stores on the sync queue, the transfer lands as the
            # gathers complete, and attention starts the moment q0 is roped.
            kv_list[0] = load_kv(0)

            # ---- q last ----
            q_tiles = [q_pool.tile([128, T], BF16, tag=f"q{h}", name=f"qt{h}")
                       for h in range(H)]
            qk_path("q", lambda mi: q_tiles[mi])

        # ================= phase B: attention ================================
        with (
            tc.tile_pool(name="probs", bufs=28) as probs_pool,
            tc.tile_pool(name="pairs", bufs=16) as pair_pool,
            tc.tile_pool(name="attm", bufs=2) as attm_pool,
            tc.tile_pool(name="attsc", bufs=2, space="PSUM") as sc_ps,
            tc.tile_pool(name="attop", bufs=1, space="PSUM") as out_ps,
            tc.tile_pool(name="attden", bufs=1, space="PSUM") as den_ps,
        ):
            att_tiles = []

            def emit_tail(st):
                """Deferred per-head softmax tail: evac, recip, bcast, mult."""
                op2, dps, ath = st
                osb = attm_pool.tile([128, 2 * QG], F32, tag="osb")
                act2(osb, op2, AF.Identity)
                dsb = attm_pool.tile([1, 2 * QG], F32, tag="dsb")
                for g in range(2):
                    nc.vector.reciprocal_approx_fast(dsb[:, GS[g]], dps[g])
                dsb_bf = attm_pool.tile([1, 2 * QG], BF16, tag="dsbb")
                nc.vector.tensor_copy(dsb_bf, dsb)
                bc2 = out_ps.tile([128, 1024], F32, tag="op")
                for g in range(2):
                    nc.tensor.matmul(bc2[:, PS2[g]], ones_row_bf,
                                     dsb_bf[:, GS[g]], start=True, stop=True)
                nc.vector.tensor_tensor(
                    ath.rearrange("p (a b) -> p a b", a=2),
                    osb.rearrange("p (a b) -> p a b", a=2),
                    bc2.rearrange("p (a b) -> p a b", a=2)[:, :, 0:QG],
                    ALU.mult)

            def pv_chunk(op2t, vwin_, prs_, ci):
                cn, pr = prs_[ci]
                vt = vwin_[:, 128 * ci:128 * (ci + 1)]
                for g in range(2):
                    nc.tensor.matmul(op2t[:, PS2[g]], vt[0:cn, :],
                                     pr[0:cn, GS[g]],
                                     start=(ci == 0), stop=(ci == N_KC - 1))

            def dps_reduce(dpst, ppart):
                for g in range(2):
                    for j, (pn, pt) in enumerate(ppart):
                        nc.tensor.matmul(dpst[g], ones_bf[0:pn, :],
                                         pt[0:pn, GS[g]],
                                         start=(j == 0), stop=(j == 1))

            def start_pv(prev_):
                """Allocate the PV accumulator + denominator for head h-1 and
                emit its first 4 PV chunks (PV leads QK by 4 in the interleave
                so the probs pool rotation never waits on a future reader)."""
                pprs, ppart, pvwin, path_ = prev_
                pop2 = out_ps.tile([128, 1024], F32, tag="op")
                pdps = [den_ps.tile([1, QG], F32, tag="dp0", name="dp0"),
                        den_ps.tile([1, QG], F32, tag="dp1", name="dp1")]
                for ci in range(4):
                    pv_chunk(pop2, pvwin, pprs, ci)
                return pop2, pdps

            prev = None          # (prs, partials, vwin, ath) of head h-1
            for h in range(H):
                ksb, vwin = kv_list[h]
                if h + 1 < H:
                    kv_list[h + 1] = load_kv(h + 1)
                ath = att_pool.tile([128, T], BF16, tag=f"att{h}")
                att_tiles.append(ath)
                if prev is not None:
                    pop2, pdps = start_pv(prev)
                # interleaved stream: this head's QK+exp chunks with the
                # previous head's PV chunks, so the PE stays busy during the
                # exp-paced QK stretch and the ACT engine never waits on PV.
                prs = []
                lvl = []
                for ci in range(N_KC):
                    c0 = 128 * ci
                    cn = min(128, 4 * T - c0)          # window is 3120 tokens
                    sp2 = sc_ps.tile([128, 1024], F32, tag="sp")
                    for g in range(2):
                        nc.tensor.matmul(
                            sp2[0:cn, PS2[g]], ksb[:, c0:c0 + cn],
                            q_tiles[h][:, GS[g]], start=True, stop=True)
                    pr = probs_pool.tile([128, 2 * QG], BF16, tag="pr")
                    act2(pr[0:cn, :], sp2[0:cn, :], AF.Exp)
                    prs.append((cn, pr))
                    # level-1 of the denominator pair tree rides the stream so
                    # the tree root is ready long before the next head's dps
                    if ci % 2 == 1 and ci < N_KC - 1:
                        pp = pair_pool.tile([128, 2 * QG], BF16, tag="pp")
                        nc.vector.tensor_tensor(pp, prs[ci - 1][1], pr,
                                                ALU.add)
                        lvl.append(pp)
                    if prev is not None:
                        if ci + 4 < N_KC:
                            pv_chunk(pop2, prev[2], prev[0], ci + 4)
                        if ci == 20:
                            dps_reduce(pdps, prev[1])
                # remaining tree levels; the 48-row tail chunk joins at the
                # matmul reduce
                while len(lvl) > 1:
                    nxt = []
                    for i in range(0, len(lvl) - 1, 2):
                        pp = pair_pool.tile([128, 2 * QG], BF16, tag="pp")
                        nc.vector.tensor_tensor(pp, lvl[i], lvl[i + 1],
                                                ALU.add)
                        nxt.append(pp)
                    if len(lvl) % 2:
                        nxt.append(lvl[-1])
                    lvl = nxt
                partials = [(128, lvl[0]), prs[N_KC - 1]]
                if prev is not None:
                    emit_tail((pop2, pdps, prev[3]))
                prev = (prs, partials, vwin, ath)
            # final head's PV has no next head to hide under; run it densely
            pop2, pdps = start_pv(prev)
            for ci in range(4, N_KC):
                pv_chunk(pop2, prev[2], prev[0], ci)
            dps_reduce(pdps, prev[1])
            emit_tail((pop2, pdps, prev[3]))

        # ================= phase C: output projection ========================
        with (
            tc.tile_pool(name="wos", bufs=3) as wo_pool,
            tc.tile_pool(name="osbp", bufs=2) as o_pool,
            tc.tile_pool(name="opps", bufs=2, space="PSUM") as op_ps,
        ):
            for od in range(DIM // 128):
                wo = wo_pool.tile([128, HD], BF16, tag="wo")
                nc.sync.dma_start(
                    wo.rearrange("p (c m) -> p c m", c=H),
                    WoT.tensor[:, 128 * od:128 * (od + 1)].rearrange(
                        "(c p) m -> p c m", p=128))
                ot = o_pool.tile([128, T], F32, tag="ot")
                ps2 = op_ps.tile([128, 1024], F32, tag="opps")
                for hc in range(H):
                    for g in range(2):
                        nc.tensor.matmul(ps2[:, PS2[g]],
                                         wo[:, 128 * hc:128 * (hc + 1)],
                                         att_tiles[hc][:, GS[g]],
                                         start=(hc == 0), stop=(hc == H - 1))
                act2(ot, ps2, AF.Identity, bias=bo_sb[:, od:od + 1])
                nc.sync.dma_start(outT.tensor[128 * od:128 * (od + 1), :], ot)


_CACHED_NC = None
_LAST_IN_MAPS = None


def _get_nc():
    global _CACHED_NC
    if _CACHED_NC is None:
        _CACHED_NC = _build_nc()
    return _CACHED_NC


def _deinterleave(n):
    """Permutation putting even rotary lanes first within each 128-dim head."""
    idx = np.arange(n).reshape(-1, D)
    return np.concatenate([idx[:, 0::2], idx[:, 1::2]], axis=1).reshape(-1)


def kernel(hidden_states, freqs_cos, freqs_sin, W_qkv, b_qkv, gq, gk, W_out,
           b_out):
    hidden_states = np.asarray(hidden_states, dtype=np.float32)
    freqs_cos = np.asarray(freqs_cos, dtype=np.float32)
    freqs_sin = np.asarray(freqs_sin, dtype=np.float32)
    W_qkv = np.asarray(W_qkv, dtype=np.float32)
    b_qkv = np.asarray(b_qkv, dtype=np.float32)
    gq = np.asarray(gq, dtype=np.float32)
    gk = np.asarray(gk, dtype=np.float32)
    W_out = np.asarray(W_out, dtype=np.float32)
    b_out = np.asarray(b_out, dtype=np.float32)

    nc = _get_nc()

    perm = _deinterleave(HD)
    Wq, Wk, Wv = W_qkv[:HD][perm], W_qkv[HD:2 * HD][perm], W_qkv[2 * HD:]
    bq, bk, bv = b_qkv[:HD][perm], b_qkv[HD:2 * HD][perm], b_qkv[2 * HD:]
    gqp, gkp = gq[perm], gk[perm]

    WqkT = np.ascontiguousarray(
        np.concatenate([Wq, Wk], axis=0).T).astype(NP_BF16)   # [1536, 3072]
    WvTa = np.concatenate([Wv.T, bv[None, :]],
                          axis=0).astype(NP_BF16)             # [1537, 1536]
    bqk_h = np.concatenate([bq, bk])
    g_h = np.concatenate([gqp, gkp])
    bqk_t = np.ascontiguousarray(bqk_h.reshape(2 * H, 128).T)       # [128, 24]
    bqkg_t = np.ascontiguousarray((bqk_h * g_h).reshape(2 * H, 128).T)
    gcol_t = np.ascontiguousarray(g_h.reshape(2 * H, 128).T)
    WoT = np.ascontiguousarray(W_out.T).astype(NP_BF16)       # [1536, 1536]
    bo = np.ascontiguousarray(b_out.reshape(DIM // 128, 128).T)  # [128, 12]

    in_maps = []
    for c in range(NCORES):
        sl = slice(c * T, (c + 1) * T)
        hidT = np.concatenate([
            np.ascontiguousarray(hidden_states[0, sl, :].T),
            np.ones((1, T), np.float32)], axis=0).astype(NP_BF16)  # [1537, 780]
        f = (c * T) // L
        if f == 0:
            win = [0, 1, 0, 1]
        else:
            base = 2 * (f - 1)
            win = [base, base + 1, base + 2, base + 3]
        cc = np.ascontiguousarray(freqs_cos[sl].T)            # [64, 780]
        ss = np.ascontiguousarray(freqs_sin[sl].T)
        csd = np.concatenate([
            np.concatenate([cc, cc], axis=0),
            np.concatenate([ss, -ss], axis=0)], axis=1).astype(NP_BF16)
        in_maps.append({
            "hidT": hidT,
            "csd": csd,
            "wink": np.asarray([[w * 3 * 128 for w in win]], np.uint32),
            "winv": np.asarray([[w * T for w in win]], np.uint32),
            "WqkT": WqkT, "WvTa": WvTa, "bqk": bqk_t, "bqkg": bqkg_t,
            "gcol": gcol_t, "WoT": WoT, "bo": bo,
        })

    global _LAST_IN_MAPS
    _LAST_IN_MAPS = in_maps
    res = bass_utils.run_bass_kernel_spmd(nc, in_maps,
                                          core_ids=list(range(NCORES)))
    out = np.empty((1, S, DIM), np.float32)
    for c in range(NCORES):
        out[0, c * T:(c + 1) * T, :] = res.results[c]["outT"].T
    return out


# revision 64
# speedup vs baseline: 1.0345x; 1.0219x over previous
"""Trainium2 Bass kernel for CausalWanSelfAttention (block-causal window attention).

Geometry: B=1, S=6240, DIM=1536, H=12 heads x D=128, frames of L=1560 tokens,
window = current + previous frame.

Sharding over 8 NeuronCores (sequence-parallel with KV AllGather):
  - core c owns tokens [780c, 780c+780): computes fused QKV for them
    (weights replicated), full-dim RMSNorm + RoPE locally,
  - AllGathers normed/roped K (feature-major [1536,780]) and V
    (token-major [780,1536]) across cores in bf16 (fp8 was tried and
    rejected: attention output is an incoherent weighted sum, so input
    quantization shows up ~1:1 in the final output),
  - attends its 780 queries to its 2-frame KV window (3120 tokens) read from
    the gathered buffers at per-core dynamic offsets. Frame-0 cores use a
    duplicated-frame window (softmax over a duplicated key set equals softmax
    over the single set exactly), so no masking is needed anywhere,
  - local output projection (all heads of a token live on one core).

Schedule: a dummy warmup collective fires first (absorbs the ~40us cold-start
of the collective firmware), then the V path (no norm -> its gathers fire
earliest), then K, then Q; all gather triggers fire in one batch after the
k rope (mid-loop triggers block the gpsimd queue's later rope swaps), and
ncfw executes them in data-ready order while attention heads unlock
progressively.

Layouts: q,k are feature-major bf16 [d, token]; v is token-major bf16 so it
is the stationary operand of the PV matmul directly. Head-dim order of q,k
is de-interleaved on the host (even rotary lanes first) so RoPE works on
contiguous partition halves.

Precision: matmul operands bf16 (fp32 PSUM accumulation); RMSNorm statistics
and softmax normalization in fp32; RoPE tables and products bf16 (2x DVE
rate). The norm gain g is applied via the ACT-engine per-partition scale
during PSUM evacuation; the per-token inverse-RMS is folded into the RoPE
cos/sin tables. Softmax denominator: exp chunks are pair-added on the Vector
engine (level 1 rides the QK stream) and reduced over partitions with
accumulating matmuls.

Attention is emitted as interleaved chunk streams: head h's QK+exp chunks
interleave with head h-1's PV chunks (PV leads by 4), so the PE stays busy
through the exp-paced QK stretch and the ACT engine never idles during PV.
Each head's softmax tail (denominator reciprocal -> PE broadcast -> final
normalize) is deferred past the next head's stream. Engine-queue placement
is deliberate: scalar = pure ACT compute; gpsimd = rope swaps, collectives,
v-window loads; sync = DRAM stores + k-window loads.
"""

import ml_dtypes
import numpy as np

import concourse.bass as bass
import concourse.bacc as bacc
import concourse.mybir as mybir
import concourse.tile as tile
from concourse import bass_utils

F32 = mybir.dt.float32
BF16 = mybir.dt.bfloat16
FP8 = mybir.dt.float8e4
U32 = mybir.dt.uint32
AF = mybir.ActivationFunctionType
ALU = mybir.AluOpType
NP_BF16 = ml_dtypes.bfloat16

# Geometry (hardcoded per the problem spec).
S, DIM, H, D = 6240, 1536, 12, 128
HD = H * D                      # 1536
L = 1560                        # frame length
NCORES = 8
T = S // NCORES                 # 780 tokens per core
QG = 390                        # query/token group: 2 per core, fits one PSUM bank
EPS = 1e-6
KQ = DIM // 128                 # 12 contraction chunks for the QKV matmuls
# token sub-tiles within a 780-token rank block: 6x128 + 1x12
TOK_SPLITS = [(i * 128, min(128, T - i * 128)) for i in range((T + 127) // 128)]
N_KC = 25                       # key chunks in the 3120-token window (24x128+48)


def _build_nc():
    nc = bacc.Bacc("TRN2", target_bir_lowering=False, debug=False,
                   enable_asserts=True, num_devices=NCORES)

    # ---- per-core inputs ----
    hidT = nc.dram_tensor("hidT", [DIM + 1, T], BF16, kind="ExternalInput").ap()
    csd = nc.dram_tensor("csd", [128, 2 * T], BF16, kind="ExternalInput").ap()
    wink = nc.dram_tensor("wink", [1, 4], U32, kind="ExternalInput").ap()  # 384*w
    winv = nc.dram_tensor("winv", [1, 4], U32, kind="ExternalInput").ap()  # 780*w

    # ---- replicated inputs ----
    WqkT = nc.dram_tensor("WqkT", [DIM, 2 * HD], BF16, kind="ExternalInput").ap()
    WvTa = nc.dram_tensor("WvTa", [DIM + 1, HD], BF16, kind="ExternalInput").ap()
    bqk = nc.dram_tensor("bqk", [128, 2 * H], F32, kind="ExternalInput").ap()
    bqkg = nc.dram_tensor("bqkg", [128, 2 * H], F32, kind="ExternalInput").ap()
    gcol = nc.dram_tensor("gcol", [128, 2 * H], F32, kind="ExternalInput").ap()
    WoT = nc.dram_tensor("WoT", [HD, DIM], BF16, kind="ExternalInput").ap()
    bo = nc.dram_tensor("bo", [128, DIM // 128], F32, kind="ExternalInput").ap()

    # ---- output (feature-major; host transposes back) ----
    outT = nc.dram_tensor("outT", [DIM, T], F32, kind="ExternalOutput").ap()

    # ---- internal DRAM for the collectives (fp8; pipelined so attention
    # heads unlock progressively) ----
    wrm = nc.dram_tensor("wrm", [1, 64], BF16)
    wrmg = nc.dram_tensor("wrmg", [NCORES, 64], BF16, addr_space="Shared")
    # vcon carries a dummy 781st row: it is written LAST (after the kcon
    # stores) for og 1/2, which delays those gathers' data-readiness so the
    # serial collective channel (which runs whatever is ready) services the
    # attention-critical K gathers first instead of the big V ones.
    kcon = [nc.dram_tensor(f"kcon{g}", [3 * 128, T], BF16) for g in range(4)]
    vcon = [nc.dram_tensor(f"vcon{o}", [T + 1, 512], BF16) for o in range(3)]
    gk = [nc.dram_tensor(f"gk{g}", [NCORES * 3 * 128, T], BF16,
                         addr_space="Shared") for g in range(4)]
    gv = [nc.dram_tensor(f"gv{o}", [NCORES * (T + 1), 512], BF16,
                         addr_space="Shared") for o in range(3)]

    with tile.TileContext(nc) as tc:
        _emit(nc, tc, hidT, csd, wink, winv, WqkT, WvTa, bqk, bqkg, gcol,
              WoT, bo, outT, kcon, vcon, gk, gv, wrm, wrmg)
    nc.compile()
    return nc


def _emit(nc, tc, hidT, csd, wink, winv, WqkT, WvTa, bqk, bqkg, gcol,
          WoT, bo, outT, kcon, vcon, gk, gv, wrm, wrmg):
    # window base registers (element offsets into gk / gv axis 0)
    kregs, vregs = [], []
    for i in range(4):
        rk = nc.alloc_registers(f"wk{i}")
        nc.regs_load(rk, wink.tensor[0:1, i:i + 1])
        kregs.append(nc.snap(rk, donate=True, min_val=0,
                             max_val=(NCORES - 1) * 3 * 128))
        rv = nc.alloc_registers(f"wv{i}")
        nc.regs_load(rv, winv.tensor[0:1, i:i + 1])
        vregs.append(nc.snap(rv, donate=True, min_val=0,
                             max_val=(NCORES - 1) * (T + 1)))

    GS = (slice(0, QG), slice(QG, 2 * QG))        # token groups in SBUF
    PS2 = (slice(0, QG), slice(512, 512 + QG))    # the two bank-aligned halves

    def act2(out_sb, ps2, func, bias=0.0, scale=1.0):
        """One ACT op over both 390-wide halves of a 2-bank PSUM tile."""
        nc.scalar.activation(
            out_sb.rearrange("p (a b) -> p a b", a=2),
            ps2.rearrange("p (a b) -> p a b", a=2)[:, :, 0:QG],
            func, bias=bias, scale=scale)

    with (
        tc.tile_pool(name="const", bufs=1) as const,
        tc.tile_pool(name="qsb", bufs=1) as q_pool,       # roped q (bf16)
        tc.tile_pool(name="attsb", bufs=1) as att_pool,   # attn out
        tc.tile_pool(name="kwin", bufs=2) as kv_pool,     # gathered k windows
        tc.tile_pool(name="vwin", bufs=3) as vt_pool,     # gathered v windows
    ):
        def load_kv(h, veng=None):
            """Issue the gathered-KV window loads for head h. ksb on sync
            (idle; the gather-completion waits must not block compute
            queues); vwin on gpsimd by default, or `veng` for the phase-A
            preload (gpsimd is blocked in the gather batch there)."""
            if veng is None:
                veng = nc.gpsimd
            ksb = kv_pool.tile([128, 4 * T], BF16, tag="ksb")
            for w in range(4):
                nc.sync.dma_start(
                    ksb[:, w * T:(w + 1) * T],
                    gk[h // 3][bass.ds(kregs[w] + 128 * (h % 3), 128), :])
            gvh = gv[h // 4]
            ho = 128 * (h % 4)
            vwin = vt_pool.tile([128, 25 * 128], BF16, tag="vwin")
            for w in range(4):
                lo = 780 * w          # window-space start of this block
                s = lo
                while s < lo + 780:
                    off = s % 128
                    if off:
                        n = min(128 - off, lo + 780 - s)
                    else:
                        n = lo + 780 - s
                    blk = s // 128
                    if off == 0 and n >= 128:
                        nb = n // 128
                        veng.dma_start(
                            vwin[:, 128 * blk:128 * (blk + nb)].rearrange(
                                "p (c d) -> p c d", d=128),
                            gvh[bass.ds(vregs[w] + (s - lo), 128 * nb),
                                ho:ho + 128].rearrange(
                                    "(c p) d -> p c d", p=128))
                        s += 128 * nb
                    else:
                        n = min(n, 128 - off)
                        veng.dma_start(
                            vwin[off:off + n, 128 * blk:128 * (blk + 1)],
                            gvh[bass.ds(vregs[w] + (s - lo), n),
                                ho:ho + 128])
                        s += n
            return ksb, vwin

        kv_list = [None] * H
        # warmup collective: absorbs the cold-start latency of the
        # collective firmware so the real gathers fire immediately.
        wu = const.tile([1, 64], BF16)
        nc.vector.memset(wu, 0.0)
        nc.gpsimd.dma_start(wrm.ap(), wu)
        nc.gpsimd.collective_compute(
            "AllGather", ALU.bypass, replica_groups=[list(range(NCORES))],
            ins=[wrm.ap()], outs=[wrmg.ap()])

        ones_col = const.tile([128, 1], F32)          # fp32 ones (norm reduce)
        nc.vector.memset(ones_col, 1.0)
        ones_bf = const.tile([128, 1], BF16)          # bf16 ones (denominator)
        nc.vector.memset(ones_bf, 1.0)
        ones_row = const.tile([1, 128], F32)          # partition-broadcast lhsT
        nc.vector.memset(ones_row, 1.0)
        ones_row_bf = const.tile([1, 128], BF16)      # bf16 broadcast lhsT
        nc.vector.memset(ones_row_bf, 1.0)
        bqk_sb = const.tile([128, 2 * H], F32)
        nc.sync.dma_start(bqk_sb, bqk)
        bqkg_sb = const.tile([128, 2 * H], F32)
        nc.sync.dma_start(bqkg_sb, bqkg)
        gcol_sb = const.tile([128, 2 * H], F32)
        nc.sync.dma_start(gcol_sb, gcol)
        bo_sb = const.tile([128, DIM // 128], F32)
        nc.sync.dma_start(bo_sb, bo)
        eps_q = const.tile([1, 1], F32)
        nc.vector.memset(eps_q, D * EPS)
        eps_k = const.tile([1, 1], F32)
        nc.vector.memset(eps_k, EPS)
        zrow = const.tile([1, 512], BF16)     # vcon dummy-row source
        nc.vector.memset(zrow, 0.0)

        # ================= phase A: QKV projections, norms, rope, gathers ====
        with (
            tc.tile_pool(name="hid", bufs=1) as hid_pool,
            tc.tile_pool(name="wls", bufs=1) as wl_pool,
            tc.tile_pool(name="vws", bufs=2) as vw_pool,
            tc.tile_pool(name="wrk", bufs=1) as wrk_pool,
            tc.tile_pool(name="kf8", bufs=2) as kf8_pool,
            tc.tile_pool(name="tmp", bufs=2) as tmp_pool,
            tc.tile_pool(name="ropet", bufs=2) as rope_pool,
            tc.tile_pool(name="small", bufs=1) as small_pool,
            tc.tile_pool(name="csp", bufs=1) as cs_pool,
            tc.tile_pool(name="qkps", bufs=3, space="PSUM") as ps_pool,
            tc.tile_pool(name="invps", bufs=1, space="PSUM") as inv_ps_pool,
            tc.tile_pool(name="redps", bufs=1, space="PSUM") as red_ps_pool,
        ):
            # DMA issue order matters: hid first (gates the first v matmul),
            # cos/sin next (small), v weights stream inside the og loop, and
            # the big q/k weight load is issued after them (needed later).
            hid_all = hid_pool.tile([128, KQ * T], BF16, tag="hid_all")
            nc.sync.dma_start(
                hid_all.rearrange("p (c t) -> p c t", c=KQ),
                hidT.tensor[0:DIM, :].rearrange("(c p) t -> p c t", p=128))
            hid = [hid_all[:, i * T:(i + 1) * T] for i in range(KQ)]
            hid_ones = hid_pool.tile([1, T], BF16, tag="hid_ones")
            nc.sync.dma_start(hid_ones, hidT.tensor[DIM:DIM + 1, :])

            # [cos;cos] in cols 0:T, [sin;-sin] in cols T:2T (bf16: rope
            # tables only scale q/k, 0.4% rounding is inside budget)
            cs_sb = cs_pool.tile([128, 2 * T], BF16)
            nc.sync.dma_start(cs_sb, csd)



            # ---- v first: token-major, contraction over dim chunks + bias row;
            # its gathers need no norm so they fire earliest. vw_pool bufs=2:
            # the next og group's weights stream in while this one computes. ----
            for og in range(HD // 512):
                vb = tmp_pool.tile([1, 512], BF16, tag="vb")
                nc.sync.dma_start(
                    vb, WvTa.tensor[DIM:DIM + 1, 512 * og:512 * (og + 1)])
                vw_all = vw_pool.tile([128, KQ * 512], BF16, tag="vw_all")
                nc.sync.dma_start(
                    vw_all.rearrange("p (c m) -> p c m", c=KQ),
                    WvTa.tensor[0:DIM, 512 * og:512 * (og + 1)].rearrange(
                        "(c p) m -> p c m", p=128))
                vw = [vw_all[:, kc * 512:(kc + 1) * 512] for kc in range(KQ)]
                for (t0, tn_) in TOK_SPLITS:
                    ps = ps_pool.tile([128, 1024], F32, tag="qkps")
                    for kc in range(KQ):
                        nc.tensor.matmul(ps[0:tn_, 0:512],
                                         hid[kc][:, t0:t0 + tn_],
                                         vw[kc], start=(kc == 0), stop=False)
                    nc.tensor.matmul(ps[0:tn_, 0:512], hid_ones[:, t0:t0 + tn_],
                                     vb, start=False, stop=True)
                    vsb = tmp_pool.tile([128, 512], BF16, tag="vsb")
                    nc.scalar.activation(vsb[0:tn_, :], ps[0:tn_, 0:512],
                                         AF.Identity)
                    nc.sync.dma_start(vcon[og].ap()[t0:t0 + tn_, :],
                                      vsb[0:tn_, :])
                if og == 0:
                    # gv0's dummy row is written now (no gating needed: it
                    # must run before anything else anyway)
                    nc.sync.dma_start(vcon[0].ap()[T:T + 1, :], zrow)
                    nc.gpsimd.collective_compute(
                        "AllGather", ALU.bypass,
                        replica_groups=[list(range(NCORES))],
                        ins=[vcon[og].ap()], outs=[gv[og].ap()])

            def qk_path(which, dest_for, chunk_done=None):
                mlo = H if which == "k" else 0
                # q-path rope swaps issue from the scalar queue: the gpsimd
                # queue at that point is blocked in the gather-trigger batch,
                # which would push the swaps (and so q0 and attention start)
                # out by ~25us. The scalar queue is idle there.
                swap_eng = nc.gpsimd if which == "k" else nc.scalar
                # per-path weight halves (heads 0-5, 6-11): each path streams
                # its own 2.3 MB while the previous compute runs, and the two
                # tags rotate so the q path's loads overlap the k projections.
                off = HD if which == "k" else 0
                HH = HD // 2
                whalves = []
                for hf in range(2):
                    wt = wl_pool.tile([128, KQ * HH], BF16, tag=f"wq{hf}")
                    nc.sync.dma_start(
                        wt.rearrange("p (c m) -> p c m", c=KQ),
                        WqkT.tensor[:, off + hf * HH:
                                    off + (hf + 1) * HH].rearrange(
                            "(c p) m -> p c m", p=128))
                    whalves.append(wt)
                # --- projection + biased/gained evac + sum of squares ---
                ssq = small_pool.tile([128, T], F32, tag="ssq")
                works = []
                for mi in range(H):
                    m = mlo + mi
                    work = wrk_pool.tile([128, T], BF16, tag=f"work{mi}",
                                          name=f"work{mi}")
                    works.append(work)
                    tsq = tmp_pool.tile([128, T], F32, tag="tsq")
                    ps2 = ps_pool.tile([128, 1024], F32, tag="qkps")
                    for kc in range(KQ):
                        wc = whalves[mi // 6][:, kc * HH + 128 * (mi % 6):
                                              kc * HH + 128 * (mi % 6 + 1)]
                        for g in range(2):
                            nc.tensor.matmul(ps2[:, PS2[g]], wc,
                                             hid[kc][:, GS[g]],
                                             start=(kc == 0),
                                             stop=(kc == KQ - 1))
                    # work = g * (x + b): scale applies before bias, so the
                    # bias table is pre-multiplied by g on the host.
                    act2(work, ps2, AF.Identity, bias=bqkg_sb[:, m:m + 1],
                         scale=gcol_sb[:, m:m + 1])
                    act2(tsq, ps2, AF.Square, bias=bqk_sb[:, m:m + 1])
                    if mi == 0:
                        nc.vector.tensor_copy(ssq, tsq)
                    else:
                        nc.vector.tensor_tensor(ssq, ssq, tsq, ALU.add)
                # --- rms scale: s = 1/sqrt(mean+eps)  (x 1/sqrt(D) for q) ---
                sq_scale = (D / DIM) if which == "q" else (1.0 / DIM)
                sq_bias = eps_q if which == "q" else eps_k
                inv = small_pool.tile([1, T], F32, tag="inv")
                rt = small_pool.tile([1, T], F32, tag="rt")
                for g in range(2):
                    red = red_ps_pool.tile([1, QG], F32, tag="redps")
                    nc.tensor.matmul(red, ones_col, ssq[:, GS[g]], start=True,
                                     stop=True)
                    nc.scalar.activation(rt[:, GS[g]], red, AF.Sqrt,
                                         bias=sq_bias, scale=sq_scale)
                nc.vector.reciprocal_approx_fast(inv, rt)
                # --- fold inv into the rope tables: one broadcast per path ---
                csi_c = small_pool.tile([128, T], BF16, tag="csic")
                csi_s = small_pool.tile([128, T], BF16, tag="csis")
                for g in range(2):
                    ibp = inv_ps_pool.tile([128, QG], F32, tag="invbc",
                                           name=f"invbc{g}")
                    nc.tensor.matmul(ibp, ones_row, inv[:, GS[g]],
                                     start=True, stop=True)
                    nc.vector.tensor_tensor(
                        csi_c[:, GS[g]], cs_sb[:, g * QG:(g + 1) * QG],
                        ibp, ALU.mult)
                    nc.vector.tensor_tensor(
                        csi_s[:, GS[g]], cs_sb[:, T + g * QG:T + (g + 1) * QG],
                        ibp, ALU.mult)
                # --- rope -> dest, full-width bf16 (2x DVE rate), per head ---
                for mi in range(H):
                    work = works[mi]
                    dest = dest_for(mi)
                    ta = rope_pool.tile([128, T], BF16, tag="ra")
                    tb = rope_pool.tile([128, T], BF16, tag="rb")
                    sw = rope_pool.tile([128, T], BF16, tag="rsw")
                    nc.vector.tensor_tensor(ta, work, csi_c, ALU.mult)
                    nc.vector.tensor_tensor(tb, work, csi_s, ALU.mult)
                    swap_eng.dma_start(sw[0:64, :], tb[64:128, :])
                    swap_eng.dma_start(sw[64:128, :], tb[0:64, :])
                    nc.vector.tensor_tensor(dest, ta, sw, ALU.add)
                    if chunk_done is not None:
                        chunk_done(mi, dest)

            # ---- k second (feeds the remaining collectives); fp8 dest
            # tiles rotate through a 4-deep pool (k lives on in DRAM) ----
            def k_dest(mi):
                return kf8_pool.tile([128, T], BF16, tag="k8", name=f"kt{mi}")

            def k_chunk_done(mi, dest):
                g = mi // 3
                nc.sync.dma_start(
                    kcon[g].ap()[128 * (mi % 3):128 * (mi % 3 + 1), :], dest)

            qk_path("k", k_dest, k_chunk_done)

            # gv1/gv2 dummy rows: emitted on sync AFTER the kcon stores, so
            # those gathers only become data-ready once K is on its way out
            # and the channel prioritizes k0/k1.
            for og in (1, 2):
                nc.sync.dma_start(vcon[og].ap()[T:T + 1, :], zrow)

            # all remaining gather triggers fire in one batch AFTER the rope
            # loop: a trigger placed mid-loop blocks the later heads' rope
            # swaps on the gpsimd queue (the input-ready wait), and ncfw
            # serializes the collectives anyway so early triggers buy nothing.
            for g in range(4):
                nc.gpsimd.collective_compute(
                    "AllGather", ALU.bypass,
                    replica_groups=[list(range(NCORES))],
                    ins=[kcon[g].ap()], outs=[gk[g].ap()])
                if g in (1, 2):     # v1 after k1, v2 after k2
                    nc.gpsimd.collective_compute(
                        "AllGather", ALU.bypass,
                        replica_groups=[list(range(NCORES))],
                        ins=[vcon[g].ap()], outs=[gv[g].ap()])

            # preload the first attention KV window: issued behind the
            # kcon stores on the sync queue, the transfer lands as the
            # gathers complete, and attention starts the moment q0 is roped.
            kv_list[0] = load_kv(0, nc.sync)

            # ---- q last ----
            q_tiles = [q_pool.tile([128, T], BF16, tag=f"q{h}", name=f"qt{h}")
                       for h in range(H)]
            qk_path("q", lambda mi: q_tiles[mi])

        # ================= phase B: attention ================================
        with (
            tc.tile_pool(name="probs", bufs=28) as probs_pool,
            tc.tile_pool(name="pairs", bufs=16) as pair_pool,
            tc.tile_pool(name="attm", bufs=2) as attm_pool,
            tc.tile_pool(name="attsc", bufs=2, space="PSUM") as sc_ps,
            tc.tile_pool(name="attop", bufs=1, space="PSUM") as out_ps,
            tc.tile_pool(name="attden", bufs=1, space="PSUM") as den_ps,
        ):
            att_tiles = []

            def emit_tail(st):
                """Deferred per-head softmax tail: evac, recip, bcast, mult."""
                op2, dps, ath = st
                osb = attm_pool.tile([128, 2 * QG], F32, tag="osb")
                act2(osb, op2, AF.Identity)
                dsb = attm_pool.tile([1, 2 * QG], F32, tag="dsb")
                for g in range(2):
                    nc.vector.reciprocal_approx_fast(dsb[:, GS[g]], dps[g])
                dsb_bf = attm_pool.tile([1, 2 * QG], BF16, tag="dsbb")
                nc.vector.tensor_copy(dsb_bf, dsb)
                bc2 = out_ps.tile([128, 1024], F32, tag="op")
                for g in range(2):
                    nc.tensor.matmul(bc2[:, PS2[g]], ones_row_bf,
                                     dsb_bf[:, GS[g]], start=True, stop=True)
                nc.vector.tensor_tensor(
                    ath.rearrange("p (a b) -> p a b", a=2),
                    osb.rearrange("p (a b) -> p a b", a=2),
                    bc2.rearrange("p (a b) -> p a b", a=2)[:, :, 0:QG],
                    ALU.mult)

            def pv_chunk(op2t, vwin_, prs_, ci):
                cn, pr = prs_[ci]
                vt = vwin_[:, 128 * ci:128 * (ci + 1)]
                for g in range(2):
                    nc.tensor.matmul(op2t[:, PS2[g]], vt[0:cn, :],
                                     pr[0:cn, GS[g]],
                                     start=(ci == 0), stop=(ci == N_KC - 1))

            def dps_reduce(dpst, ppart):
                for g in range(2):
                    for j, (pn, pt) in enumerate(ppart):
                        nc.tensor.matmul(dpst[g], ones_bf[0:pn, :],
                                         pt[0:pn, GS[g]],
                                         start=(j == 0), stop=(j == 1))

            def start_pv(prev_):
                """Allocate the PV accumulator + denominator for head h-1 and
                emit its first 4 PV chunks (PV leads QK by 4 in the interleave
                so the probs pool rotation never waits on a future reader)."""
                pprs, ppart, pvwin, path_ = prev_
                pop2 = out_ps.tile([128, 1024], F32, tag="op")
                pdps = [den_ps.tile([1, QG], F32, tag="dp0", name="dp0"),
                        den_ps.tile([1, QG], F32, tag="dp1", name="dp1")]
                for ci in range(4):
                    pv_chunk(pop2, pvwin, pprs, ci)
                return pop2, pdps

            prev = None          # (prs, partials, vwin, ath) of head h-1
            for h in range(H):
                ksb, vwin = kv_list[h]
                if h + 1 < H:
                    kv_list[h + 1] = load_kv(h + 1)
                ath = att_pool.tile([128, T], BF16, tag=f"att{h}")
                att_tiles.append(ath)
                if prev is not None:
                    pop2, pdps = start_pv(prev)
                # interleaved stream: this head's QK+exp chunks with the
                # previous head's PV chunks, so the PE stays busy during the
                # exp-paced QK stretch and the ACT engine never waits on PV.
                prs = []
                lvl = []
                for ci in range(N_KC):
                    c0 = 128 * ci
                    cn = min(128, 4 * T - c0)          # window is 3120 tokens
                    sp2 = sc_ps.tile([128, 1024], F32, tag="sp")
                    for g in range(2):
                        nc.tensor.matmul(
                            sp2[0:cn, PS2[g]], ksb[:, c0:c0 + cn],
                            q_tiles[h][:, GS[g]], start=True, stop=True)
                    pr = probs_pool.tile([128, 2 * QG], BF16, tag="pr")
                    act2(pr[0:cn, :], sp2[0:cn, :], AF.Exp)
                    prs.append((cn, pr))
                    # level-1 of the denominator pair tree rides the stream so
                    # the tree root is ready long before the next head's dps
                    if ci % 2 == 1 and ci < N_KC - 1:
                        pp = pair_pool.tile([128, 2 * QG], BF16, tag="pp")
                        nc.vector.tensor_tensor(pp, prs[ci - 1][1], pr,
                                                ALU.add)
                        lvl.append(pp)
                    if prev is not None:
                        if ci + 4 < N_KC:
                            pv_chunk(pop2, prev[2], prev[0], ci + 4)
                        if ci == 20:
                            dps_reduce(pdps, prev[1])
                # remaining tree levels; the 48-row tail chunk joins at the
                # matmul reduce
                while len(lvl) > 1:
                    nxt = []
                    for i in range(0, len(lvl) - 1, 2):
                        pp = pair_pool.tile([128, 2 * QG], BF16, tag="pp")
                        nc.vector.tensor_tensor(pp, lvl[i], lvl[i + 1],
                                                ALU.add)
                        nxt.append(pp)
                    if len(lvl) % 2:
                        nxt.append(lvl[-1])
                    lvl = nxt
                partials = [(128, lvl[0]), prs[N_KC - 1]]
                if prev is not None:
                    emit_tail((pop2, pdps, prev[3]))
                prev = (prs, partials, vwin, ath)
            # final head's PV has no next head to hide under; run it densely
            pop2, pdps = start_pv(prev)
            for ci in range(4, N_KC):
                pv_chunk(pop2, prev[2], prev[0], ci)
            dps_reduce(pdps, prev[1])
            emit_tail((pop2, pdps, prev[3]))

        # ================= phase C: output projection ========================
        with (
            tc.tile_pool(name="wos", bufs=3) as wo_pool,
            tc.tile_pool(name="osbp", bufs=2) as o_pool,
            tc.tile_pool(name="opps", bufs=2, space="PSUM") as op_ps,
        ):
            for od in range(DIM // 128):
                wo = wo_pool.tile([128, HD], BF16, tag="wo")
                nc.sync.dma_start(
                    wo.rearrange("p (c m) -> p c m", c=H),
                    WoT.tensor[:, 128 * od:128 * (od + 1)].rearrange(
                        "(c p) m -> p c m", p=128))
                ot = o_pool.tile([128, T], F32, tag="ot")
                ps2 = op_ps.tile([128, 1024], F32, tag="opps")
                for hc in range(H):
                    for g in range(2):
                        nc.tensor.matmul(ps2[:, PS2[g]],
                                         wo[:, 128 * hc:128 * (hc + 1)],
                                         att_tiles[hc][:, GS[g]],
                                         start=(hc == 0), stop=(hc == H - 1))
                act2(ot, ps2, AF.Identity, bias=bo_sb[:, od:od + 1])
                nc.sync.dma_start(outT.tensor[128 * od:128 * (od + 1), :], ot)


_CACHED_NC = None
_LAST_IN_MAPS = None


def _get_nc():
    global _CACHED_NC
    if _CACHED_NC is None:
        _CACHED_NC = _build_nc()
    return _CACHED_NC


def _deinterleave(n):
    """Permutation putting even rotary lanes first within each 128-dim head."""
    idx = np.arange(n).reshape(-1, D)
    return np.concatenate([idx[:, 0::2], idx[:, 1::2]], axis=1).reshape(-1)


def kernel(hidden_states, freqs_cos, freqs_sin, W_qkv, b_qkv, gq, gk, W_out,
           b_out):
    hidden_states = np.asarray(hidden_states, dtype=np.float32)
    freqs_cos = np.asarray(freqs_cos, dtype=np.float32)
    freqs_sin = np.asarray(freqs_sin, dtype=np.float32)
    W_qkv = np.asarray(W_qkv, dtype=np.float32)
    b_qkv = np.asarray(b_qkv, dtype=np.float32)
    gq = np.asarray(gq, dtype=np.float32)
    gk = np.asarray(gk, dtype=np.float32)
    W_out = np.asarray(W_out, dtype=np.float32)
    b_out = np.asarray(b_out, dtype=np.float32)

    nc = _get_nc()

    perm = _deinterleave(HD)
    Wq, Wk, Wv = W_qkv[:HD][perm], W_qkv[HD:2 * HD][perm], W_qkv[2 * HD:]
    bq, bk, bv = b_qkv[:HD][perm], b_qkv[HD:2 * HD][perm], b_qkv[2 * HD:]
    gqp, gkp = gq[perm], gk[perm]

    WqkT = np.ascontiguousarray(
        np.concatenate([Wq, Wk], axis=0).T).astype(NP_BF16)   # [1536, 3072]
    WvTa = np.concatenate([Wv.T, bv[None, :]],
                          axis=0).astype(NP_BF16)             # [1537, 1536]
    bqk_h = np.concatenate([bq, bk])
    g_h = np.concatenate([gqp, gkp])
    bqk_t = np.ascontiguousarray(bqk_h.reshape(2 * H, 128).T)       # [128, 24]
    bqkg_t = np.ascontiguousarray((bqk_h * g_h).reshape(2 * H, 128).T)
    gcol_t = np.ascontiguousarray(g_h.reshape(2 * H, 128).T)
    WoT = np.ascontiguousarray(W_out.T).astype(NP_BF16)       # [1536, 1536]
    bo = np.ascontiguousarray(b_out.reshape(DIM // 128, 128).T)  # [128, 12]

    in_maps = []
    for c in range(NCORES):
        sl = slice(c * T, (c + 1) * T)
        hidT = np.concatenate([
            np.ascontiguousarray(hidden_states[0, sl, :].T),
            np.ones((1, T), np.float32)], axis=0).astype(NP_BF16)  # [1537, 780]
        f = (c * T) // L
        if f == 0:
            win = [0, 1, 0, 1]
        else:
            base = 2 * (f - 1)
            win = [base, base + 1, base + 2, base + 3]
        cc = np.ascontiguousarray(freqs_cos[sl].T)            # [64, 780]
        ss = np.ascontiguousarray(freqs_sin[sl].T)
        csd = np.concatenate([
            np.concatenate([cc, cc], axis=0),
            np.concatenate([ss, -ss], axis=0)], axis=1).astype(NP_BF16)
        in_maps.append({
            "hidT": hidT,
            "csd": csd,
            "wink": np.asarray([[w * 3 * 128 for w in win]], np.uint32),
            "winv": np.asarray([[w * (T + 1) for w in win]], np.uint32),
            "WqkT": WqkT, "WvTa": WvTa, "bqk": bqk_t, "bqkg": bqkg_t,
            "gcol": gcol_t, "WoT": WoT, "bo": bo,
        })

    global _LAST_IN_MAPS
    _LAST_IN_MAPS = in_maps
    res = bass_utils.run_bass_kernel_spmd(nc, in_maps,
                                          core_ids=list(range(NCORES)))
    out = np.empty((1, S, DIM), np.float32)
    for c in range(NCORES):
        out[0, c * T:(c + 1) * T, :] = res.results[c]["outT"].T
    return out
